# revision 2
# baseline (speedup 1.0000x reference)
"""AFNO Trainium2 kernel v2 — stage-major, complex-packed DFT matmuls.

Per core (one batch element), 8 channel blocks processed as 4 pairs:
  S1  packed-real W-DFT, blockdiag pair lhsT [128,128], out tile1 [w'|w', c, (yr h|yi h)]
  T1  xbar: -> s2rhs [(yr h|yi h), c, (A w'|B w')]
  S2  complex H-DFT one matmul [[Ch,-Sh],[Sh,Ch]] -> (zr h'|zi h') full 128x128
  T2  -> tile3 [c, w', (zr|zi)]
  MLP block-diag complex 2-layer (relu, softshrink)
  T3  -> t3o [(r2 h'|i2 h'), w', c]
  S4  complex inverse-H, pair-packed out (A h|B h)
  T4  -> s5rhs [(vr w'|vi w'), c, (A h|B h)]
  S5  packed inverse-W -> out [w, c, (A h|B h)]
Bias path (Conv1d k=1 GEMM) issued as 8 filler groups between spectral
stages to keep the PE continuously busy (p-state ramp: full clock only
after 3us of uninterrupted tensor work). Final spectral+bias add on host.
"""

import sys, os
for p in ("/opt/trn_rl_repo", "/root/.axon_site/_ro/trn_rl_repo"):
    if os.path.isdir(p) and p not in sys.path:
        sys.path.insert(0, p)

import numpy as np
import ml_dtypes
from contextlib import ExitStack

import concourse.bass as bass
from concourse import bacc
import concourse.mybir as mybir
import concourse.tile as tile
from concourse.bass import ts
from concourse.bass_utils import run_bass_kernel_spmd

BF16 = mybir.dt.bfloat16
F32 = mybir.dt.float32
NPBF16 = ml_dtypes.bfloat16
AF = mybir.ActivationFunctionType
ALU = mybir.AluOpType

DIM, H, W, NB = 768, 64, 64, 8
BS = DIM // NB     # 96
LAMBD = 0.01
NCORES = 8
WCHUNKS = [(0, 8), (8, 8), (16, 8), (24, 8), (32, 1)]      # MLP w' chunks
CCHUNKS = [(0, 15), (15, 15), (30, 15), (45, 15), (60, 15), (75, 15), (90, 6)]
PCHUNKS = [(0, 5), (5, 5), (10, 5), (15, 5), (20, 5), (25, 5), (30, 3)]


def _build_mats():
    w = np.arange(W, dtype=np.float64)
    h = np.arange(H, dtype=np.float64)
    wp = np.arange(33, dtype=np.float64)
    cosw = np.cos(2 * np.pi * np.outer(w, wp) / 64) / 64.0
    sinw = -np.sin(2 * np.pi * np.outer(w, wp) / 64) / 64.0
    s1a = np.zeros((128, 128))
    s1b = np.zeros((128, 128))
    s1a[0:64, 0:33] = cosw
    s1a[64:128, 64:97] = cosw
    s1b[0:64, 0:33] = sinw
    s1b[64:128, 64:97] = sinw

    ang = 2 * np.pi * np.outer(h, h) / 64
    Ch, Sh = np.cos(ang), np.sin(ang)
    s2 = np.zeros((128, 128))
    s2[0:64, 0:64] = Ch
    s2[64:128, 0:64] = Sh
    s2[0:64, 64:128] = -Sh
    s2[64:128, 64:128] = Ch

    s4al = np.zeros((128, 128)); s4ah = np.zeros((128, 128))
    s4bl = np.zeros((128, 128)); s4bh = np.zeros((128, 128))
    s4al[0:64, 0:64] = Ch;   s4al[64:128, 0:64] = -Sh
    s4ah[0:64, 64:128] = Ch; s4ah[64:128, 64:128] = -Sh
    s4bl[0:64, 0:64] = Sh;   s4bl[64:128, 0:64] = Ch
    s4bh[0:64, 64:128] = Sh; s4bh[64:128, 64:128] = Ch

    s5 = np.zeros((128, 64))
    s5[0, :] = 1.0 / 64
    for k in range(1, 32):
        s5[k, :] = 2 * np.cos(2 * np.pi * w * k / 64) / 64
    s5[32, :] = np.cos(np.pi * w) / 64
    for k in range(1, 32):
        s5[64 + k, :] = -2 * np.sin(2 * np.pi * w * k / 64) / 64

    return {n: a.astype(np.float32).astype(NPBF16) for n, a in
            [("s1a", s1a), ("s1b", s1b), ("s2m", s2), ("s4al", s4al),
             ("s4ah", s4ah), ("s4bl", s4bl), ("s4bh", s4bh), ("s5m", s5)]}


def build_nc():
    nc = bacc.Bacc("TRN2", target_bir_lowering=False, debug=False)

    xs_d = nc.declare_dram_parameter("xs", [4, 128, BS, H], BF16, isOutput=False)
    xt_d = nc.declare_dram_parameter("xt", [8, 6, 128, 512], BF16, isOutput=False)
    wbt_d = nc.declare_dram_parameter("wbt", [6, 128, DIM], BF16, isOutput=False)
    ospec_d = nc.declare_dram_parameter("out_spec", [4, 12, 2, W, 4, 128], BF16,
                                        isOutput=True)
    obias_d = nc.declare_dram_parameter("out_bias", [6, 128, 8, 512], BF16,
                                        isOutput=True)

    cds = {}
    for n in ["s1a", "s1b", "s2m", "s4al", "s4ah", "s4bl", "s4bh"]:
        cds[n] = nc.declare_dram_parameter(n, [128, 128], BF16, isOutput=False)
    cds["s5m"] = nc.declare_dram_parameter("s5m", [128, 64], BF16, isOutput=False)
    for n in ["w1r", "w1i", "w1in", "w2r", "w2i", "w2in"]:
        cds[n] = nc.declare_dram_parameter(n, [BS, NB * BS], BF16, isOutput=False)
    for n in ["b1r", "b1i", "a1r", "a2r", "a1i", "a2i"]:
        cds[n] = nc.declare_dram_parameter(n, [BS, NB], F32, isOutput=False)

    with ExitStack() as ctx:
        tc = ctx.enter_context(tile.TileContext(nc))

        cpool = ctx.enter_context(tc.tile_pool(name="consts", bufs=1))
        xsp = ctx.enter_context(tc.tile_pool(name="xsp", bufs=1))
        xtp = ctx.enter_context(tc.tile_pool(name="xtp", bufs=2))
        bigp = ctx.enter_context(tc.tile_pool(name="bigp", bufs=2))
        t2p = ctx.enter_context(tc.tile_pool(name="t2p", bufs=1))
        t3p = ctx.enter_context(tc.tile_pool(name="t3p", bufs=1))
        r1p = ctx.enter_context(tc.tile_pool(name="r1p", bufs=2))
        t5p = ctx.enter_context(tc.tile_pool(name="t5p", bufs=1))
        t3op = ctx.enter_context(tc.tile_pool(name="t3op", bufs=2))
        smp = ctx.enter_context(tc.tile_pool(name="smp", bufs=1))
        stp = ctx.enter_context(tc.tile_pool(name="stp", bufs=2))
        pp = ctx.enter_context(tc.tile_pool(name="ps", bufs=6, space="PSUM"))
        ppb = ctx.enter_context(tc.tile_pool(name="psb", bufs=2, space="PSUM"))

        # bias-path weights first (gpsimd) so the warmup filler starts ASAP;
        # spectral consts go on the idle sync queue in parallel
        wbt_t = []
        for kk in range(6):
            t = cpool.tile([128, DIM], BF16, tag=f"c_wbt{kk}", name=f"c_wbt{kk}")
            nc.gpsimd.dma_start(t[:], wbt_d[kk])
            wbt_t.append(t)
        ct = {}
        for n in ["s1a", "s1b", "s2m", "s4al", "s4ah", "s4bl", "s4bh"]:
            ct[n] = cpool.tile([128, 128], BF16, tag=f"c_{n}", name=f"c_{n}")
            nc.sync.dma_start(ct[n][:], cds[n][:])
        ct["s5m"] = cpool.tile([128, 64], BF16, tag="c_s5m", name="c_s5m")
        nc.sync.dma_start(ct["s5m"][:], cds["s5m"][:])
        for n in ["w1r", "w1i", "w1in", "w2r", "w2i", "w2in"]:
            ct[n] = cpool.tile([BS, NB * BS], BF16, tag=f"c_{n}", name=f"c_{n}")
            nc.sync.dma_start(ct[n][:], cds[n][:])
        for n in ["b1r", "b1i", "a1r", "a2r", "a1i", "a2i"]:
            ct[n] = cpool.tile([BS, NB], F32, tag=f"c_{n}", name=f"c_{n}")
            nc.sync.dma_start(ct[n][:], cds[n][:])

        # engine rotation for PSUM->SBUF copies (gpsimd cannot access PSUM);
        # vector-weighted since scalar also runs the MLP activations
        cp_engines = [nc.vector.tensor_copy, nc.scalar.copy,
                      nc.vector.tensor_copy]
        _cpi = [0]

        def cp(*args):
            f = cp_engines[_cpi[0] % 3]
            _cpi[0] += 1
            return f(*args)

        # ---- bias filler half-group: token-chunk t, 3 out-chunks (~3.9us) ----
        xt_tiles = {}

        def xt_load(t):
            xt_t = xtp.tile([128, 6, 512], BF16, tag="xt", name=f"xt{t}")
            nc.gpsimd.dma_start(xt_t[:], xt_d[t].rearrange("k p f -> p k f"))
            xt_tiles[t] = xt_t

        def bias_half(t, hi):
            xt_t = xt_tiles[t]
            bst = stp.tile([128, 2, 512], BF16, tag="bst")
            for i in range(3):
                oc = 3 * hi + i
                pb = ppb.tile([128, 512], F32, tag="psb")
                for kk in range(6):
                    nc.tensor.matmul(pb[:], wbt_t[kk][:, ts(oc, 128)],
                                     xt_t[:, kk, :], start=(kk == 0),
                                     stop=(kk == 5))
                if i < 2:
                    cp(bst[:, i, :], pb[:])
                else:
                    bst2 = stp.tile([128, 512], BF16, tag="bst2")
                    cp(bst2[:], pb[:])
                    nc.gpsimd.dma_start(obias_d[oc, :, t, :], bst2[:])
            # one DMA for the first two out-chunks of this half-group
            nc.gpsimd.dma_start(
                obias_d.rearrange("a p b c -> p a b c")[:, 3 * hi:3 * hi + 2, t, :],
                bst[:])
            if hi:
                del xt_tiles[t]
                if t + 2 < 8:
                    xt_load(t + 2)

        def s1_pair(p):
            xs_t = xsp.tile([128, BS, H], BF16, tag="xs", name=f"xs{p}")
            nc.gpsimd.dma_start(xs_t[:], xs_d[p])
            t1 = bigp.tile([128, BS, 128], BF16, tag="t1p", name=f"t1p{p}")
            s2rhs = bigp.tile([128, BS, 128], BF16, tag="t1o", name=f"s2r{p}")
            # c-halves: emit each half's transposes as soon as its copies land
            for c0 in (0, 48):
                for ci in range(c0 // 8, c0 // 8 + 6):
                    psA = pp.tile([128, 512], F32, tag="ps")
                    nc.tensor.matmul(psA[:], ct["s1a"][:],
                                     xs_t[:, 8 * ci:8 * ci + 8, :],
                                     start=True, stop=True)
                    psB = pp.tile([128, 512], F32, tag="ps")
                    nc.tensor.matmul(psB[:], ct["s1b"][:],
                                     xs_t[:, 8 * ci:8 * ci + 8, :],
                                     start=True, stop=True)
                    cp(t1[:, 8 * ci:8 * ci + 8, 0:64],
                       psA.rearrange("p (a b) -> p a b", a=8))
                    cp(t1[:, 8 * ci:8 * ci + 8, 64:128],
                       psB.rearrange("p (a b) -> p a b", a=8))
                nc.sync.dma_start_transpose(
                    s2rhs[:, c0:c0 + 48, 0:48],
                    t1[0:48, c0:c0 + 48, :].rearrange("p a b -> p (a b)"))
                nc.sync.dma_start_transpose(
                    s2rhs[:, c0:c0 + 48, 64:112],
                    t1[64:112, c0:c0 + 48, :].rearrange("p a b -> p (a b)"))
            return s2rhs

        def s2_block(s2rhs, sl, j):
            # w'-major chunks; T2 part1 (w' 0:16) issued as soon as its
            # chunks land so the MLP's first chunks unblock early
            tile2 = t2p.tile([128, 33, 128], BF16, tag="t2", name=f"t2_{j}")
            nc.gpsimd.memset(tile2[:, :, 96:128], 0.0)
            tile3 = t3p.tile([128, 33, 128], BF16, tag="t3", name=f"t3_{j}")
            for (w0, ww) in PCHUNKS:
                ps = pp.tile([128, 5 * BS], F32, tag="ps")
                nc.tensor.matmul(ps[:, :ww * BS], ct["s2m"][:],
                                 s2rhs[:, 0:96, sl + w0:sl + w0 + ww],
                                 start=True, stop=True)
                cp(tile2[:, w0:w0 + ww, 0:96],
                   ps[:, :ww * BS].rearrange("p (a b) -> p b a", a=96))
                if w0 == 15:
                    nc.sync.dma_start_transpose(
                        tile3[:, 0:16, :],
                        tile2[:, 0:16, :].rearrange("p a b -> p (a b)"))
            nc.sync.dma_start_transpose(
                tile3[:, 16:33, :],
                tile2[:, 16:33, :].rearrange("p a b -> p (a b)"))
            return tile3

        def mlp_l1(tile3, j):
            r1 = r1p.tile([BS, 33, H], BF16, tag="r1", name=f"r1_{j}")
            i1 = r1p.tile([BS, 33, H], BF16, tag="i1", name=f"i1_{j}")
            for (w0, wc) in WCHUNKS:
                n = wc * H
                rr = tile3[0:96, w0:w0 + wc, 0:64]
                ri = tile3[0:96, w0:w0 + wc, 64:128]
                pr = pp.tile([BS, 512], F32, tag="ps")
                pi = pp.tile([BS, 512], F32, tag="ps")
                nc.tensor.matmul(pr[:, :n], ct["w1r"][:, ts(j, BS)], rr,
                                 start=True, stop=False)
                nc.tensor.matmul(pr[:, :n], ct["w1in"][:, ts(j, BS)], ri,
                                 start=False, stop=True)
                nc.tensor.matmul(pi[:, :n], ct["w1i"][:, ts(j, BS)], rr,
                                 start=True, stop=False)
                nc.tensor.matmul(pi[:, :n], ct["w1r"][:, ts(j, BS)], ri,
                                 start=False, stop=True)
                nc.vector.tensor_scalar(r1[:, w0:w0 + wc, :], pr[:, :n],
                                        ct["b1r"][:, j:j + 1], 0.0,
                                        ALU.add, ALU.max)
                nc.scalar.activation(i1[:, w0:w0 + wc, :], pi[:, :n],
                                     AF.Relu, bias=ct["b1i"][:, j:j + 1])
            return r1, i1

        def mlp_l2(r1, i1, j):
            tile5 = t5p.tile([BS, 33, 128], BF16, tag="t5", name=f"t5_{j}")
            for (w0, wc) in WCHUNKS:
                n = wc * H
                rr, ri = r1[:, w0:w0 + wc, :], i1[:, w0:w0 + wc, :]
                pr = pp.tile([BS, 512], F32, tag="ps")
                pi = pp.tile([BS, 512], F32, tag="ps")
                nc.tensor.matmul(pr[:, :n], ct["w2r"][:, ts(j, BS)], rr,
                                 start=True, stop=False)
                nc.tensor.matmul(pr[:, :n], ct["w2in"][:, ts(j, BS)], ri,
                                 start=False, stop=True)
                nc.tensor.matmul(pi[:, :n], ct["w2i"][:, ts(j, BS)], rr,
                                 start=True, stop=False)
                nc.tensor.matmul(pi[:, :n], ct["w2r"][:, ts(j, BS)], ri,
                                 start=False, stop=True)
                sa = smp.tile([BS, 2, 512], BF16, tag="sar")
                sb = smp.tile([BS, 2, 512], BF16, tag="sbr")
                nc.scalar.activation(sa[:, 0, :n], pr[:, :n], AF.Relu,
                                     bias=ct["a1r"][:, j:j + 1])
                nc.scalar.activation(sb[:, 0, :n], pr[:, :n], AF.Relu,
                                     bias=ct["a2r"][:, j:j + 1], scale=-1.0)
                nc.scalar.activation(sa[:, 1, :n], pi[:, :n], AF.Relu,
                                     bias=ct["a1i"][:, j:j + 1])
                nc.scalar.activation(sb[:, 1, :n], pi[:, :n], AF.Relu,
                                     bias=ct["a2i"][:, j:j + 1], scale=-1.0)
                eng = nc.vector if (w0 % 16 == 0) else nc.gpsimd
                eng.tensor_sub(
                    tile5[:, w0:w0 + wc, :],
                    sa[:, :, :n].rearrange("p g (a b) -> p a g b", a=wc),
                    sb[:, :, :n].rearrange("p g (a b) -> p a g b", a=wc))
            return tile5

        def t3_transpose(j, tile5):
            t3o = t3op.tile([128, 33, BS], BF16, tag="t3o", name=f"t3o_{j}")
            nc.sync.dma_start_transpose(
                t3o[:, 0:16, :],
                tile5[:, 0:16, :].rearrange("p a b -> p (a b)"))
            nc.sync.dma_start_transpose(
                t3o[:, 16:33, :],
                tile5[:, 16:33, :].rearrange("p a b -> p (a b)"))
            return t3o

        def s4_compute(p, t3oA, t3oB):
            # c-chunks, c-half-major; each T4 half issued as soon as its
            # half of tile6 is complete
            tile6 = bigp.tile([128, BS, 128], BF16, tag="t1p", name=f"t6_{p}")
            s5rhs = bigp.tile([128, BS, 128], BF16, tag="t1o", name=f"s5r{p}")
            nc.gpsimd.memset(tile6[:, :, 33:64], 0.0)
            nc.gpsimd.memset(tile6[:, :, 97:128], 0.0)
            for ch0 in (0, 48):
                for la, lb, moff in (("s4al", "s4ah", 0), ("s4bl", "s4bh", 64)):
                    for c0 in range(ch0, ch0 + 48, 12):
                        ps = pp.tile([128, 33 * 12], F32, tag="ps")
                        nc.tensor.matmul(ps[:], ct[la][:],
                                         t3oA[:, :, c0:c0 + 12],
                                         start=True, stop=False)
                        nc.tensor.matmul(ps[:], ct[lb][:],
                                         t3oB[:, :, c0:c0 + 12],
                                         start=False, stop=True)
                        cp(tile6[:, c0:c0 + 12, moff:moff + 33],
                           ps.rearrange("p (a b) -> p b a", a=33))
                nc.sync.dma_start_transpose(
                    s5rhs[:, ch0:ch0 + 48, :],
                    tile6[:, ch0:ch0 + 48, :].rearrange("p a b -> p (a b)"))
            return s5rhs

        def s5_pair(p, s5rhs):
            # chunk pairs in opposite PE column quadrants; one copy + one
            # store per [128, 512] double-chunk
            for cc2 in range(12):
                ps = pp.tile([128, 512], F32, tag="ps")
                nc.tensor.matmul(ps[0:64, :], ct["s5m"][:],
                                 s5rhs[:, 8 * cc2:8 * cc2 + 4, :],
                                 start=True, stop=True)
                nc.tensor.matmul(ps[64:128, :], ct["s5m"][:],
                                 s5rhs[:, 8 * cc2 + 4:8 * cc2 + 8, :],
                                 start=True, stop=True)
                st = stp.tile([128, 512], BF16, tag="s5st")
                cp(st[:], ps[:])
                # alternate store queues so the drain tail isn't serialized
                # on gpsimd (plain dma_start on scalar is safe; only the
                # xbar transpose corrupts there)
                st_eng = nc.gpsimd if (cc2 % 2 == 0) else nc.scalar
                st_eng.dma_start(
                    ospec_d[p, cc2].rearrange("e w f hp -> (e w) (f hp)"),
                    st[:])

        # ================== schedule (software pipeline) ==================
        # Helper state
        s2r = {}    # pair -> s2rhs
        t3l = {}    # block -> tile3
        r1l = {}    # block -> (r1, i1)
        t5l = {}    # block -> tile5
        t3o = {}    # block -> t3o
        t6l = {}    # pair -> tile6
        s5r = {}    # pair -> s5rhs

        xt_load(0)
        xt_load(1)
        bias_half(0, 0)
        bias_half(0, 1)
        bias_half(1, 0)

        s2r[0] = s1_pair(0)
        bias_half(1, 1)
        s2r[1] = s1_pair(1)
        t3l[0] = s2_block(s2r[0], 0, 0)
        t3l[1] = s2_block(s2r[0], 64, 1)
        r1l[0] = mlp_l1(t3l[0], 0)
        s2r[2] = s1_pair(2)
        t3l[2] = s2_block(s2r[1], 0, 2)
        r1l[1] = mlp_l1(t3l[1], 1)
        t5l[0] = mlp_l2(*r1l[0], 0)
        t3o[0] = t3_transpose(0, t5l[0])
        bias_half(2, 0)
        t3l[3] = s2_block(s2r[1], 64, 3)
        r1l[2] = mlp_l1(t3l[2], 2)
        t5l[1] = mlp_l2(*r1l[1], 1)
        t3o[1] = t3_transpose(1, t5l[1])
        s2r[3] = s1_pair(3)
        bias_half(2, 1)
        t3l[4] = s2_block(s2r[2], 0, 4)
        r1l[3] = mlp_l1(t3l[3], 3)
        t5l[2] = mlp_l2(*r1l[2], 2)
        s5r[0] = s4_compute(0, t3o[0], t3o[1])
        t3o[2] = t3_transpose(2, t5l[2])
        bias_half(3, 0)
        t3l[5] = s2_block(s2r[2], 64, 5)
        r1l[4] = mlp_l1(t3l[4], 4)
        t5l[3] = mlp_l2(*r1l[3], 3)
        t3o[3] = t3_transpose(3, t5l[3])
        bias_half(3, 1)
        t3l[6] = s2_block(s2r[3], 0, 6)
        r1l[5] = mlp_l1(t3l[5], 5)
        t5l[4] = mlp_l2(*r1l[4], 4)
        s5r[1] = s4_compute(1, t3o[2], t3o[3])
        t3o[4] = t3_transpose(4, t5l[4])
        bias_half(4, 0)
        s5_pair(0, s5r[0])
        t3l[7] = s2_block(s2r[3], 64, 7)
        r1l[6] = mlp_l1(t3l[6], 6)
        t5l[5] = mlp_l2(*r1l[5], 5)
        t3o[5] = t3_transpose(5, t5l[5])
        bias_half(4, 1)
        r1l[7] = mlp_l1(t3l[7], 7)
        t5l[6] = mlp_l2(*r1l[6], 6)
        s5r[2] = s4_compute(2, t3o[4], t3o[5])
        t3o[6] = t3_transpose(6, t5l[6])
        bias_half(5, 0)
        s5_pair(1, s5r[1])
        bias_half(5, 1)
        t5l[7] = mlp_l2(*r1l[7], 7)
        t3o[7] = t3_transpose(7, t5l[7])
        bias_half(6, 0)
        s5r[3] = s4_compute(3, t3o[6], t3o[7])
        bias_half(6, 1)
        s5_pair(2, s5r[2])
        bias_half(7, 0)
        s5_pair(3, s5r[3])
        bias_half(7, 1)

    return nc


_nc_cache = None


def _get_nc():
    global _nc_cache
    if _nc_cache is None:
        _nc_cache = build_nc()
        _nc_cache.finalize()
    return _nc_cache


def make_in_maps(x, w1, b1, w2, b2, Wb, bb):
    shared = dict(_build_mats())
    shared["w1r"] = np.concatenate([w1[0][b] for b in range(NB)], 1).astype(NPBF16)
    shared["w1i"] = np.concatenate([w1[1][b] for b in range(NB)], 1).astype(NPBF16)
    shared["w1in"] = np.concatenate([-w1[1][b] for b in range(NB)], 1).astype(NPBF16)
    shared["w2r"] = np.concatenate([w2[0][b] for b in range(NB)], 1).astype(NPBF16)
    shared["w2i"] = np.concatenate([w2[1][b] for b in range(NB)], 1).astype(NPBF16)
    shared["w2in"] = np.concatenate([-w2[1][b] for b in range(NB)], 1).astype(NPBF16)
    shared["b1r"] = np.ascontiguousarray(b1[0].T).astype(np.float32)
    shared["b1i"] = np.ascontiguousarray(b1[1].T).astype(np.float32)
    shared["a1r"] = np.ascontiguousarray((b2[0] - LAMBD).T).astype(np.float32)
    shared["a2r"] = np.ascontiguousarray((-b2[0] - LAMBD).T).astype(np.float32)
    shared["a1i"] = np.ascontiguousarray((b2[1] - LAMBD).T).astype(np.float32)
    shared["a2i"] = np.ascontiguousarray((-b2[1] - LAMBD).T).astype(np.float32)
    shared["wbt"] = np.ascontiguousarray(Wb.T.reshape(6, 128, DIM)).astype(NPBF16)
    # (s4a/s4b/s5m/s1a/s1b/s2m come from _build_mats)

    in_maps = []
    for b in range(NCORES):
        m = dict(shared)
        xg = x[b].reshape(H, W, DIM)
        xs = np.empty((4, 128, BS, H), np.float32)
        for p in range(4):
            xs[p, 0:64] = xg[:, :, (2 * p) * BS:(2 * p + 1) * BS].transpose(1, 2, 0)
            xs[p, 64:128] = xg[:, :, (2 * p + 1) * BS:(2 * p + 2) * BS].transpose(1, 2, 0)
        m["xs"] = xs.astype(NPBF16)
        m["xt"] = np.ascontiguousarray(
            x[b].T.reshape(6, 128, 8, 512).transpose(2, 0, 1, 3)).astype(NPBF16)
        in_maps.append(m)
    return in_maps


def _assemble(res_b):
    # out_spec [p, cc2, e, w, f, hp] ; c = 8*cc2 + 4*e + f ; hp = (q, h)
    o = np.asarray(res_b["out_spec"], np.float32)
    o = o.reshape(4, 12, 2, W, 4, 2, 64)                # [p,cc2,e,w,f,q,h]
    # -> [h, w, p, q, (cc2 e f)=c]
    spec = o.transpose(6, 3, 0, 5, 1, 2, 4).reshape(H * W, DIM)
    bia = np.asarray(res_b["out_bias"], np.float32).reshape(6, 128, 8, 512)
    bia = bia.transpose(2, 3, 0, 1).reshape(H * W, DIM)
    return spec, bia


def kernel(x, w1, b1, w2, b2, Wb, bb, _trace=False):
    nc = _get_nc()
    bb = np.asarray(bb, np.float32)
    in_maps = make_in_maps(np.asarray(x, np.float32), np.asarray(w1, np.float32),
                           np.asarray(b1, np.float32), np.asarray(w2, np.float32),
                           np.asarray(b2, np.float32), np.asarray(Wb, np.float32),
                           bb)
    res = run_bass_kernel_spmd(nc, in_maps, list(range(NCORES)), trace=_trace)
    outs = []
    for b in range(NCORES):
        spec, bia = _assemble(res.results[b])
        outs.append(spec + bia + bb[None, :])
    full = np.stack(outs, axis=0)
    if _trace:
        return full, res
    return full


# revision 4
# speedup vs baseline: 1.0348x; 1.0348x over previous
"""AFNO Trainium2 kernel v2 — stage-major, complex-packed DFT matmuls.

Per core (one batch element), 8 channel blocks processed as 4 pairs:
  S1  packed-real W-DFT, blockdiag pair lhsT [128,128], out tile1 [w'|w', c, (yr h|yi h)]
  T1  xbar: -> s2rhs [(yr h|yi h), c, (A w'|B w')]
  S2  complex H-DFT one matmul [[Ch,-Sh],[Sh,Ch]] -> (zr h'|zi h') full 128x128
  T2  -> tile3 [c, w', (zr|zi)]
  MLP block-diag complex 2-layer (relu, softshrink)
  T3  -> t3o [(r2 h'|i2 h'), w', c]
  S4  complex inverse-H, pair-packed out (A h|B h)
  T4  -> s5rhs [(vr w'|vi w'), c, (A h|B h)]
  S5  packed inverse-W -> out [w, c, (A h|B h)]
Bias path (Conv1d k=1 GEMM) issued as 8 filler groups between spectral
stages to keep the PE continuously busy (p-state ramp: full clock only
after 3us of uninterrupted tensor work). Final spectral+bias add on host.
"""

import sys, os
for p in ("/opt/trn_rl_repo", "/root/.axon_site/_ro/trn_rl_repo"):
    if os.path.isdir(p) and p not in sys.path:
        sys.path.insert(0, p)

import numpy as np
import ml_dtypes
from contextlib import ExitStack

import concourse.bass as bass
from concourse import bacc
import concourse.mybir as mybir
import concourse.tile as tile
from concourse.bass import ts
from concourse.bass_utils import run_bass_kernel_spmd

BF16 = mybir.dt.bfloat16
F32 = mybir.dt.float32
NPBF16 = ml_dtypes.bfloat16
AF = mybir.ActivationFunctionType
ALU = mybir.AluOpType

DIM, H, W, NB = 768, 64, 64, 8
BS = DIM // NB     # 96
LAMBD = 0.01
NCORES = 8
WCHUNKS = [(0, 8), (8, 8), (16, 8), (24, 8), (32, 1)]      # MLP w' chunks
CCHUNKS = [(0, 15), (15, 15), (30, 15), (45, 15), (60, 15), (75, 15), (90, 6)]
PCHUNKS = [(0, 5), (5, 5), (10, 5), (15, 5), (20, 5), (25, 5), (30, 3)]


def _build_mats():
    w = np.arange(W, dtype=np.float64)
    h = np.arange(H, dtype=np.float64)
    wp = np.arange(33, dtype=np.float64)
    cosw = np.cos(2 * np.pi * np.outer(w, wp) / 64) / 64.0
    sinw = -np.sin(2 * np.pi * np.outer(w, wp) / 64) / 64.0
    s1a = np.zeros((128, 128))
    s1b = np.zeros((128, 128))
    s1a[0:64, 0:33] = cosw
    s1a[64:128, 64:97] = cosw
    s1b[0:64, 0:33] = sinw
    s1b[64:128, 64:97] = sinw

    ang = 2 * np.pi * np.outer(h, h) / 64
    Ch, Sh = np.cos(ang), np.sin(ang)
    s2 = np.zeros((128, 128))
    s2[0:64, 0:64] = Ch
    s2[64:128, 0:64] = Sh
    s2[0:64, 64:128] = -Sh
    s2[64:128, 64:128] = Ch

    s4al = np.zeros((128, 128)); s4ah = np.zeros((128, 128))
    s4bl = np.zeros((128, 128)); s4bh = np.zeros((128, 128))
    s4al[0:64, 0:64] = Ch;   s4al[64:128, 0:64] = -Sh
    s4ah[0:64, 64:128] = Ch; s4ah[64:128, 64:128] = -Sh
    s4bl[0:64, 0:64] = Sh;   s4bl[64:128, 0:64] = Ch
    s4bh[0:64, 64:128] = Sh; s4bh[64:128, 64:128] = Ch

    s5 = np.zeros((128, 64))
    s5[0, :] = 1.0 / 64
    for k in range(1, 32):
        s5[k, :] = 2 * np.cos(2 * np.pi * w * k / 64) / 64
    s5[32, :] = np.cos(np.pi * w) / 64
    for k in range(1, 32):
        s5[64 + k, :] = -2 * np.sin(2 * np.pi * w * k / 64) / 64

    return {n: a.astype(np.float32).astype(NPBF16) for n, a in
            [("s1a", s1a), ("s1b", s1b), ("s2m", s2), ("s4al", s4al),
             ("s4ah", s4ah), ("s4bl", s4bl), ("s4bh", s4bh), ("s5m", s5)]}


def build_nc():
    nc = bacc.Bacc("TRN2", target_bir_lowering=False, debug=False)

    xs_d = nc.declare_dram_parameter("xs", [4, 128, BS, H], BF16, isOutput=False)
    xt_d = nc.declare_dram_parameter("xt", [8, 6, 128, 512], BF16, isOutput=False)
    wbt_d = nc.declare_dram_parameter("wbt", [6, 128, DIM], BF16, isOutput=False)
    ospec_d = nc.declare_dram_parameter("out_spec", [4, 12, 2, W, 4, 128], BF16,
                                        isOutput=True)
    obias_d = nc.declare_dram_parameter("out_bias", [6, 128, 8, 512], BF16,
                                        isOutput=True)

    cds = {}
    for n in ["s1a", "s1b", "s2m", "s4al", "s4ah", "s4bl", "s4bh"]:
        cds[n] = nc.declare_dram_parameter(n, [128, 128], BF16, isOutput=False)
    cds["s5m"] = nc.declare_dram_parameter("s5m", [128, 64], BF16, isOutput=False)
    for n in ["w1r", "w1i", "w1in", "w2r", "w2i", "w2in"]:
        cds[n] = nc.declare_dram_parameter(n, [BS, NB * BS], BF16, isOutput=False)
    for n in ["b1r", "b1i", "a1r", "a2r", "a1i", "a2i"]:
        cds[n] = nc.declare_dram_parameter(n, [BS, NB], F32, isOutput=False)

    with ExitStack() as ctx:
        tc = ctx.enter_context(tile.TileContext(nc))

        cpool = ctx.enter_context(tc.tile_pool(name="consts", bufs=1))
        xsp = ctx.enter_context(tc.tile_pool(name="xsp", bufs=1))
        xtp = ctx.enter_context(tc.tile_pool(name="xtp", bufs=2))
        bigp = ctx.enter_context(tc.tile_pool(name="bigp", bufs=2))
        t2p = ctx.enter_context(tc.tile_pool(name="t2p", bufs=1))
        t3p = ctx.enter_context(tc.tile_pool(name="t3p", bufs=1))
        r1p = ctx.enter_context(tc.tile_pool(name="r1p", bufs=2))
        t5p = ctx.enter_context(tc.tile_pool(name="t5p", bufs=1))
        t3op = ctx.enter_context(tc.tile_pool(name="t3op", bufs=2))
        smp = ctx.enter_context(tc.tile_pool(name="smp", bufs=1))
        stp = ctx.enter_context(tc.tile_pool(name="stp", bufs=2))
        pp = ctx.enter_context(tc.tile_pool(name="ps", bufs=6, space="PSUM"))
        ppb = ctx.enter_context(tc.tile_pool(name="psb", bufs=2, space="PSUM"))

        # bias-path weights first (gpsimd) so the warmup filler starts ASAP;
        # spectral consts go on the idle sync queue in parallel
        wbt_t = []
        for kk in range(6):
            t = cpool.tile([128, DIM], BF16, tag=f"c_wbt{kk}", name=f"c_wbt{kk}")
            nc.gpsimd.dma_start(t[:], wbt_d[kk])
            wbt_t.append(t)
        ct = {}
        for n in ["s1a", "s1b", "s2m", "s4al", "s4ah", "s4bl", "s4bh"]:
            ct[n] = cpool.tile([128, 128], BF16, tag=f"c_{n}", name=f"c_{n}")
            nc.sync.dma_start(ct[n][:], cds[n][:])
        ct["s5m"] = cpool.tile([128, 64], BF16, tag="c_s5m", name="c_s5m")
        nc.sync.dma_start(ct["s5m"][:], cds["s5m"][:])
        for n in ["w1r", "w1i", "w1in", "w2r", "w2i", "w2in"]:
            ct[n] = cpool.tile([BS, NB * BS], BF16, tag=f"c_{n}", name=f"c_{n}")
            nc.sync.dma_start(ct[n][:], cds[n][:])
        for n in ["b1r", "b1i", "a1r", "a2r", "a1i", "a2i"]:
            ct[n] = cpool.tile([BS, NB], F32, tag=f"c_{n}", name=f"c_{n}")
            nc.sync.dma_start(ct[n][:], cds[n][:])

        # engine rotation for PSUM->SBUF copies (gpsimd cannot access PSUM);
        # vector-weighted since scalar also runs the MLP activations
        cp_engines = [nc.scalar.copy, nc.vector.tensor_copy,
                      nc.scalar.copy]
        _cpi = [0]

        def cp(*args):
            f = cp_engines[_cpi[0] % 3]
            _cpi[0] += 1
            return f(*args)

        # ---- bias filler half-group: token-chunk t, 3 out-chunks (~3.9us) ----
        xt_tiles = {}

        def xt_load(t):
            xt_t = xtp.tile([128, 6, 512], BF16, tag="xt", name=f"xt{t}")
            nc.sync.dma_start(xt_t[:], xt_d[t].rearrange("k p f -> p k f"))
            xt_tiles[t] = xt_t

        def bias_half(t, hi):
            xt_t = xt_tiles[t]
            bst = stp.tile([128, 2, 512], BF16, tag="bst")
            for i in range(3):
                oc = 3 * hi + i
                pb = ppb.tile([128, 512], F32, tag="psb")
                for kk in range(6):
                    nc.tensor.matmul(pb[:], wbt_t[kk][:, ts(oc, 128)],
                                     xt_t[:, kk, :], start=(kk == 0),
                                     stop=(kk == 5))
                if i < 2:
                    cp(bst[:, i, :], pb[:])
                else:
                    bst2 = stp.tile([128, 512], BF16, tag="bst2")
                    cp(bst2[:], pb[:])
                    nc.gpsimd.dma_start(obias_d[oc, :, t, :], bst2[:])
            # one DMA for the first two out-chunks of this half-group
            nc.gpsimd.dma_start(
                obias_d.rearrange("a p b c -> p a b c")[:, 3 * hi:3 * hi + 2, t, :],
                bst[:])
            if hi:
                del xt_tiles[t]
                if t + 2 < 8:
                    xt_load(t + 2)

        def s1_pair(p):
            xs_t = xsp.tile([128, BS, H], BF16, tag="xs", name=f"xs{p}")
            nc.gpsimd.dma_start(xs_t[:], xs_d[p])
            t1 = bigp.tile([128, BS, 128], BF16, tag="t1p", name=f"t1p{p}")
            s2rhs = bigp.tile([128, BS, 128], BF16, tag="t1o", name=f"s2r{p}")
            # c-halves: emit each half's transposes as soon as its copies land
            for c0 in (0, 48):
                for ci in range(c0 // 8, c0 // 8 + 6):
                    psA = pp.tile([128, 512], F32, tag="ps")
                    nc.tensor.matmul(psA[:], ct["s1a"][:],
                                     xs_t[:, 8 * ci:8 * ci + 8, :],
                                     start=True, stop=True)
                    psB = pp.tile([128, 512], F32, tag="ps")
                    nc.tensor.matmul(psB[:], ct["s1b"][:],
                                     xs_t[:, 8 * ci:8 * ci + 8, :],
                                     start=True, stop=True)
                    cp(t1[:, 8 * ci:8 * ci + 8, 0:64],
                       psA.rearrange("p (a b) -> p a b", a=8))
                    cp(t1[:, 8 * ci:8 * ci + 8, 64:128],
                       psB.rearrange("p (a b) -> p a b", a=8))
                nc.sync.dma_start_transpose(
                    s2rhs[:, c0:c0 + 48, 0:48],
                    t1[0:48, c0:c0 + 48, :].rearrange("p a b -> p (a b)"))
                nc.sync.dma_start_transpose(
                    s2rhs[:, c0:c0 + 48, 64:112],
                    t1[64:112, c0:c0 + 48, :].rearrange("p a b -> p (a b)"))
            return s2rhs

        def s2_block(s2rhs, sl, j):
            # w'-major chunks; T2 part1 (w' 0:16) issued as soon as its
            # chunks land so the MLP's first chunks unblock early
            tile2 = t2p.tile([128, 33, 128], BF16, tag="t2", name=f"t2_{j}")
            nc.gpsimd.memset(tile2[:, :, 96:128], 0.0)
            tile3 = t3p.tile([128, 33, 128], BF16, tag="t3", name=f"t3_{j}")
            for (w0, ww) in PCHUNKS:
                ps = pp.tile([128, 5 * BS], F32, tag="ps")
                nc.tensor.matmul(ps[:, :ww * BS], ct["s2m"][:],
                                 s2rhs[:, 0:96, sl + w0:sl + w0 + ww],
                                 start=True, stop=True)
                cp(tile2[:, w0:w0 + ww, 0:96],
                   ps[:, :ww * BS].rearrange("p (a b) -> p b a", a=96))
                if w0 == 15:
                    nc.sync.dma_start_transpose(
                        tile3[:, 0:16, :],
                        tile2[:, 0:16, :].rearrange("p a b -> p (a b)"))
            nc.sync.dma_start_transpose(
                tile3[:, 16:33, :],
                tile2[:, 16:33, :].rearrange("p a b -> p (a b)"))
            return tile3

        def mlp_l1(tile3, j):
            r1 = r1p.tile([BS, 33, H], BF16, tag="r1", name=f"r1_{j}")
            i1 = r1p.tile([BS, 33, H], BF16, tag="i1", name=f"i1_{j}")
            for (w0, wc) in WCHUNKS:
                n = wc * H
                rr = tile3[0:96, w0:w0 + wc, 0:64]
                ri = tile3[0:96, w0:w0 + wc, 64:128]
                pr = pp.tile([BS, 512], F32, tag="ps")
                pi = pp.tile([BS, 512], F32, tag="ps")
                nc.tensor.matmul(pr[:, :n], ct["w1r"][:, ts(j, BS)], rr,
                                 start=True, stop=False)
                nc.tensor.matmul(pr[:, :n], ct["w1in"][:, ts(j, BS)], ri,
                                 start=False, stop=True)
                nc.tensor.matmul(pi[:, :n], ct["w1i"][:, ts(j, BS)], rr,
                                 start=True, stop=False)
                nc.tensor.matmul(pi[:, :n], ct["w1r"][:, ts(j, BS)], ri,
                                 start=False, stop=True)
                nc.vector.tensor_scalar(r1[:, w0:w0 + wc, :], pr[:, :n],
                                        ct["b1r"][:, j:j + 1], 0.0,
                                        ALU.add, ALU.max)
                nc.scalar.activation(i1[:, w0:w0 + wc, :], pi[:, :n],
                                     AF.Relu, bias=ct["b1i"][:, j:j + 1])
            return r1, i1

        def mlp_l2(r1, i1, j):
            tile5 = t5p.tile([BS, 33, 128], BF16, tag="t5", name=f"t5_{j}")
            for (w0, wc) in WCHUNKS:
                n = wc * H
                rr, ri = r1[:, w0:w0 + wc, :], i1[:, w0:w0 + wc, :]
                pr = pp.tile([BS, 512], F32, tag="ps")
                pi = pp.tile([BS, 512], F32, tag="ps")
                nc.tensor.matmul(pr[:, :n], ct["w2r"][:, ts(j, BS)], rr,
                                 start=True, stop=False)
                nc.tensor.matmul(pr[:, :n], ct["w2in"][:, ts(j, BS)], ri,
                                 start=False, stop=True)
                nc.tensor.matmul(pi[:, :n], ct["w2i"][:, ts(j, BS)], rr,
                                 start=True, stop=False)
                nc.tensor.matmul(pi[:, :n], ct["w2r"][:, ts(j, BS)], ri,
                                 start=False, stop=True)
                sa = smp.tile([BS, 2, 512], BF16, tag="sar")
                sb = smp.tile([BS, 2, 512], BF16, tag="sbr")
                nc.vector.tensor_scalar(sa[:, 0, :n], pr[:, :n],
                                        ct["a1r"][:, j:j + 1], 0.0,
                                        ALU.add, ALU.max)
                nc.scalar.activation(sb[:, 0, :n], pr[:, :n], AF.Relu,
                                     bias=ct["a2r"][:, j:j + 1], scale=-1.0)
                nc.vector.tensor_scalar(sa[:, 1, :n], pi[:, :n],
                                        ct["a1i"][:, j:j + 1], 0.0,
                                        ALU.add, ALU.max)
                nc.scalar.activation(sb[:, 1, :n], pi[:, :n], AF.Relu,
                                     bias=ct["a2i"][:, j:j + 1], scale=-1.0)
                eng = nc.vector if (w0 % 16 == 0) else nc.gpsimd
                eng.tensor_sub(
                    tile5[:, w0:w0 + wc, :],
                    sa[:, :, :n].rearrange("p g (a b) -> p a g b", a=wc),
                    sb[:, :, :n].rearrange("p g (a b) -> p a g b", a=wc))
            return tile5

        def t3_transpose(j, tile5):
            t3o = t3op.tile([128, 33, BS], BF16, tag="t3o", name=f"t3o_{j}")
            nc.sync.dma_start_transpose(
                t3o[:, 0:16, :],
                tile5[:, 0:16, :].rearrange("p a b -> p (a b)"))
            nc.sync.dma_start_transpose(
                t3o[:, 16:33, :],
                tile5[:, 16:33, :].rearrange("p a b -> p (a b)"))
            return t3o

        def s4_compute(p, t3oA, t3oB):
            # c-chunks, c-half-major; each T4 half issued as soon as its
            # half of tile6 is complete
            tile6 = bigp.tile([128, BS, 128], BF16, tag="t1p", name=f"t6_{p}")
            s5rhs = bigp.tile([128, BS, 128], BF16, tag="t1o", name=f"s5r{p}")
            nc.gpsimd.memset(tile6[:, :, 33:64], 0.0)
            nc.gpsimd.memset(tile6[:, :, 97:128], 0.0)
            for ch0 in (0, 48):
                for la, lb, moff in (("s4al", "s4ah", 0), ("s4bl", "s4bh", 64)):
                    for c0 in range(ch0, ch0 + 48, 12):
                        ps = pp.tile([128, 33 * 12], F32, tag="ps")
                        nc.tensor.matmul(ps[:], ct[la][:],
                                         t3oA[:, :, c0:c0 + 12],
                                         start=True, stop=False)
                        nc.tensor.matmul(ps[:], ct[lb][:],
                                         t3oB[:, :, c0:c0 + 12],
                                         start=False, stop=True)
                        cp(tile6[:, c0:c0 + 12, moff:moff + 33],
                           ps.rearrange("p (a b) -> p b a", a=33))
                nc.sync.dma_start_transpose(
                    s5rhs[:, ch0:ch0 + 48, :],
                    tile6[:, ch0:ch0 + 48, :].rearrange("p a b -> p (a b)"))
            return s5rhs

        def s5_pair(p, s5rhs):
            # chunk pairs in opposite PE column quadrants; one copy + one
            # store per [128, 512] double-chunk
            for cc2 in range(12):
                ps = pp.tile([128, 512], F32, tag="ps")
                nc.tensor.matmul(ps[0:64, :], ct["s5m"][:],
                                 s5rhs[:, 8 * cc2:8 * cc2 + 4, :],
                                 start=True, stop=True)
                nc.tensor.matmul(ps[64:128, :], ct["s5m"][:],
                                 s5rhs[:, 8 * cc2 + 4:8 * cc2 + 8, :],
                                 start=True, stop=True)
                st = stp.tile([128, 512], BF16, tag="s5st")
                cp(st[:], ps[:])
                # alternate store queues so the drain tail isn't serialized
                # on gpsimd (plain dma_start on scalar is safe; only the
                # xbar transpose corrupts there)
                st_eng = nc.gpsimd if (cc2 % 2 == 0) else nc.scalar
                st_eng.dma_start(
                    ospec_d[p, cc2].rearrange("e w f hp -> (e w) (f hp)"),
                    st[:])

        # ================== schedule (software pipeline) ==================
        # Helper state
        s2r = {}    # pair -> s2rhs
        t3l = {}    # block -> tile3
        r1l = {}    # block -> (r1, i1)
        t5l = {}    # block -> tile5
        t3o = {}    # block -> t3o
        t6l = {}    # pair -> tile6
        s5r = {}    # pair -> s5rhs

        xt_load(0)
        xt_load(1)
        bias_half(0, 0)
        bias_half(0, 1)
        bias_half(1, 0)

        s2r[0] = s1_pair(0)
        bias_half(1, 1)
        s2r[1] = s1_pair(1)
        t3l[0] = s2_block(s2r[0], 0, 0)
        t3l[1] = s2_block(s2r[0], 64, 1)
        r1l[0] = mlp_l1(t3l[0], 0)
        s2r[2] = s1_pair(2)
        t3l[2] = s2_block(s2r[1], 0, 2)
        r1l[1] = mlp_l1(t3l[1], 1)
        t5l[0] = mlp_l2(*r1l[0], 0)
        t3o[0] = t3_transpose(0, t5l[0])
        bias_half(2, 0)
        t3l[3] = s2_block(s2r[1], 64, 3)
        r1l[2] = mlp_l1(t3l[2], 2)
        t5l[1] = mlp_l2(*r1l[1], 1)
        t3o[1] = t3_transpose(1, t5l[1])
        s2r[3] = s1_pair(3)
        bias_half(2, 1)
        t3l[4] = s2_block(s2r[2], 0, 4)
        r1l[3] = mlp_l1(t3l[3], 3)
        t5l[2] = mlp_l2(*r1l[2], 2)
        s5r[0] = s4_compute(0, t3o[0], t3o[1])
        t3o[2] = t3_transpose(2, t5l[2])
        bias_half(3, 0)
        t3l[5] = s2_block(s2r[2], 64, 5)
        r1l[4] = mlp_l1(t3l[4], 4)
        t5l[3] = mlp_l2(*r1l[3], 3)
        t3o[3] = t3_transpose(3, t5l[3])
        bias_half(3, 1)
        t3l[6] = s2_block(s2r[3], 0, 6)
        r1l[5] = mlp_l1(t3l[5], 5)
        t5l[4] = mlp_l2(*r1l[4], 4)
        s5r[1] = s4_compute(1, t3o[2], t3o[3])
        t3o[4] = t3_transpose(4, t5l[4])
        bias_half(4, 0)
        s5_pair(0, s5r[0])
        t3l[7] = s2_block(s2r[3], 64, 7)
        r1l[6] = mlp_l1(t3l[6], 6)
        t5l[5] = mlp_l2(*r1l[5], 5)
        t3o[5] = t3_transpose(5, t5l[5])
        bias_half(4, 1)
        r1l[7] = mlp_l1(t3l[7], 7)
        t5l[6] = mlp_l2(*r1l[6], 6)
        s5r[2] = s4_compute(2, t3o[4], t3o[5])
        t3o[6] = t3_transpose(6, t5l[6])
        bias_half(5, 0)
        s5_pair(1, s5r[1])
        bias_half(5, 1)
        t5l[7] = mlp_l2(*r1l[7], 7)
        t3o[7] = t3_transpose(7, t5l[7])
        bias_half(6, 0)
        s5r[3] = s4_compute(3, t3o[6], t3o[7])
        bias_half(6, 1)
        s5_pair(2, s5r[2])
        bias_half(7, 0)
        s5_pair(3, s5r[3])
        bias_half(7, 1)

    return nc


_nc_cache = None


def _get_nc():
    global _nc_cache
    if _nc_cache is None:
        _nc_cache = build_nc()
        _nc_cache.finalize()
    return _nc_cache


def make_in_maps(x, w1, b1, w2, b2, Wb, bb):
    shared = dict(_build_mats())
    shared["w1r"] = np.concatenate([w1[0][b] for b in range(NB)], 1).astype(NPBF16)
    shared["w1i"] = np.concatenate([w1[1][b] for b in range(NB)], 1).astype(NPBF16)
    shared["w1in"] = np.concatenate([-w1[1][b] for b in range(NB)], 1).astype(NPBF16)
    shared["w2r"] = np.concatenate([w2[0][b] for b in range(NB)], 1).astype(NPBF16)
    shared["w2i"] = np.concatenate([w2[1][b] for b in range(NB)], 1).astype(NPBF16)
    shared["w2in"] = np.concatenate([-w2[1][b] for b in range(NB)], 1).astype(NPBF16)
    shared["b1r"] = np.ascontiguousarray(b1[0].T).astype(np.float32)
    shared["b1i"] = np.ascontiguousarray(b1[1].T).astype(np.float32)
    shared["a1r"] = np.ascontiguousarray((b2[0] - LAMBD).T).astype(np.float32)
    shared["a2r"] = np.ascontiguousarray((-b2[0] - LAMBD).T).astype(np.float32)
    shared["a1i"] = np.ascontiguousarray((b2[1] - LAMBD).T).astype(np.float32)
    shared["a2i"] = np.ascontiguousarray((-b2[1] - LAMBD).T).astype(np.float32)
    shared["wbt"] = np.ascontiguousarray(Wb.T.reshape(6, 128, DIM)).astype(NPBF16)
    # (s4a/s4b/s5m/s1a/s1b/s2m come from _build_mats)

    in_maps = []
    for b in range(NCORES):
        m = dict(shared)
        xg = x[b].reshape(H, W, DIM)
        xs = np.empty((4, 128, BS, H), np.float32)
        for p in range(4):
            xs[p, 0:64] = xg[:, :, (2 * p) * BS:(2 * p + 1) * BS].transpose(1, 2, 0)
            xs[p, 64:128] = xg[:, :, (2 * p + 1) * BS:(2 * p + 2) * BS].transpose(1, 2, 0)
        m["xs"] = xs.astype(NPBF16)
        m["xt"] = np.ascontiguousarray(
            x[b].T.reshape(6, 128, 8, 512).transpose(2, 0, 1, 3)).astype(NPBF16)
        in_maps.append(m)
    return in_maps


def _assemble(res_b):
    # out_spec [p, cc2, e, w, f, hp] ; c = 8*cc2 + 4*e + f ; hp = (q, h)
    o = np.asarray(res_b["out_spec"], np.float32)
    o = o.reshape(4, 12, 2, W, 4, 2, 64)                # [p,cc2,e,w,f,q,h]
    # -> [h, w, p, q, (cc2 e f)=c]
    spec = o.transpose(6, 3, 0, 5, 1, 2, 4).reshape(H * W, DIM)
    bia = np.asarray(res_b["out_bias"], np.float32).reshape(6, 128, 8, 512)
    bia = bia.transpose(2, 3, 0, 1).reshape(H * W, DIM)
    return spec, bia


def kernel(x, w1, b1, w2, b2, Wb, bb, _trace=False):
    nc = _get_nc()
    bb = np.asarray(bb, np.float32)
    in_maps = make_in_maps(np.asarray(x, np.float32), np.asarray(w1, np.float32),
                           np.asarray(b1, np.float32), np.asarray(w2, np.float32),
                           np.asarray(b2, np.float32), np.asarray(Wb, np.float32),
                           bb)
    res = run_bass_kernel_spmd(nc, in_maps, list(range(NCORES)), trace=_trace)
    outs = []
    for b in range(NCORES):
        spec, bia = _assemble(res.results[b])
        outs.append(spec + bia + bb[None, :])
    full = np.stack(outs, axis=0)
    if _trace:
        return full, res
    return full
